# revision 24
# baseline (speedup 1.0000x reference)
"""MoE layer (N=4096, D=1024, H=4096, E=8, top-2) on 8 Trainium2 cores.

Strategy: hidden-dim tensor-parallel.
  - Host computes the small gate, top-2 ids and softmax weights, groups the
    8192 (token, expert) pairs by expert, and replicates the grouped
    activations to all 8 cores.
  - Core c holds the hidden slice [c*512, (c+1)*512) of ALL experts' W1/W2
    (SBUF-resident, loaded once) and computes for every pair the partial
    FFN over its slice; host sums the 8 partial outputs, adds b2 and does
    the gate-weighted scatter into the [N, D] output.
  - Every core runs the exact same pair columns, so there is zero capacity
    padding and perfect load balance regardless of routing skew.

Device kernel (identical SPMD program on all 8 cores):
  - All matmul operands fp16 (bf16 PE rate, fp32 PSUM accumulation).
  - Pairs processed in expert-pure chunks of <=512 columns:
      gemm1: h[hm][:, chunk]  = relu(sum_dk W1t.T @ x)     (ACT drains PSUM)
      gemm2: yT[dc][:, chunk] = sum_hk W2t.T @ h           (DVE drains PSUM)
  - Ring plan (v2): ALL input streams (x groups + weights) ride the sync
    HWDGE ring; ALL y writes ride the scalar HWDGE ring.  gpsimd/SWDGE is
    not used for any steady-state DMA: its descriptor rings share SBUF AXI
    ports with SDMA engines 7/15, and a single straggling per-engine
    completion increment on the y semaphore was observed to stall the PE
    ~2.7us mid-kernel and re-throttle the HAM clock gate.
  - Startup ramp: the first expert processed has ~1024 pairs split
    [192, 320, rest]; its W1/W2 load as two half-tiles each so the first
    gemm1/gemm2 chains can start on the first half while the second is in
    flight (chunk-0 gemm1 runs dk 0-3 for an hm pair, then dk 4-7).  The
    first expert's W2 block is packed dc-major so gemm2 dc 0-3 only needs
    the first half.
  - Drain tail: the last expert ends with a 96-wide chunk; every chunk's y
    is written as two half DMAs (dc 0-3 after their casts, dc 4-7 after)
    so the post-compute drain is short.
"""

import numpy as np

from concourse import bacc
import concourse.mybir as mybir
from concourse.tile import TileContext
import concourse.bass_utils as bass_utils

N_TOK, D, H, E, TOPK = 4096, 1024, 4096, 8, 2
NCORES = 8
PAIRS = N_TOK * TOPK  # 8192 (token, expert) pairs, expert-grouped
HS = H // NCORES      # 512 hidden units per core
DK = D // 128         # 8 contraction tiles for gemm1
HMT = HS // 128       # 4 hidden tiles (gemm1 out / gemm2 contraction)
DCT = D // 128        # 8 output-column tiles for gemm2
CHUNK = 512           # max pair-chunk width (one fp32 PSUM bank)
XGW = 880             # max packed-x-group width (SBUF budget cap)
WARM_MM = 10          # HAM warmup matmuls bridging entry -> first data
LEAD = (256, 320)     # startup ramp chunk widths for the first expert
TAIL_W = 96           # width of the final drain-friendly chunk

TRACE = False
TRACE_CORES = None
LAST_RESULTS = None

_NC_CACHE = {}


def _expert_order(counts):
    """First expert: count closest to LEAD-ramp-friendly 1024; last: smallest
    count (gets the TAIL_W drain chunk); middle: natural order."""
    first = min(range(E), key=lambda e: abs(counts[e] - 1024))
    rest = [e for e in range(E) if e != first]
    last = min(rest, key=lambda e: counts[e])
    mid = [e for e in rest if e != last]
    return [first] + mid + [last]


def _balanced(n, wmax=CHUNK):
    k = -(-n // wmax) if n else 0
    return [n // k + (1 if i < n % k else 0) for i in range(k)]


def _chunks(counts):
    """Expert-pure chunks of width <= CHUNK, with a startup ramp on the
    first expert and a short drain chunk on the last."""
    order = _expert_order(counts)
    out = []
    off = 0
    for i, e in enumerate(order):
        n = counts[e]
        if i == 0 and n > sum(LEAD) + 64:
            ws = list(LEAD) + _balanced(n - sum(LEAD))
        elif i == len(order) - 1 and n > TAIL_W + 128:
            ws = _balanced(n - TAIL_W) + [TAIL_W]
        else:
            ws = _balanced(n)
        for w in ws:
            out.append((e, off, w))
            off += w
    return out


def _xgroups(chunks):
    """Chunk groups for packed x descriptors: [c0], [c1], then greedy pairs
    capped at XGW total width (SBUF budget)."""
    groups = [(0, 1)]
    if len(chunks) > 1:
        groups.append((1, 1))
    i = 2
    while i < len(chunks):
        n = 1
        if i + 1 < len(chunks) and chunks[i][2] + chunks[i + 1][2] <= XGW:
            n = 2
        groups.append((i, n))
        i += n
    return groups


def _build_nc(counts):
    f16, f32 = mybir.dt.float16, mybir.dt.float32
    Relu = mybir.ActivationFunctionType.Relu
    nc = bacc.Bacc("TRN2", target_bir_lowering=False)
    xP = nc.dram_tensor("xP", [128, DK * PAIRS], f16, kind="ExternalInput")
    W1 = nc.dram_tensor("W1", [E * 128, DK * HS], f16, kind="ExternalInput")
    W2 = nc.dram_tensor("W2", [E * 128, HMT * D], f16, kind="ExternalInput")
    b1 = nc.dram_tensor("b1", [128, E * HMT], f32, kind="ExternalInput")
    yP = nc.dram_tensor("yP", [128, DCT * PAIRS], f16, kind="ExternalOutput")

    order = _expert_order(counts)
    e_first = order[0]
    chunks = _chunks(counts)
    groups = _xgroups(chunks)
    n_chunks = len(chunks)
    xgrp_max = max(sum(chunks[c0 + j][2] for j in range(ng)) for c0, ng in groups)
    WHALF = DK * HS // 2  # columns per W1/W2 half tile

    def w2tile(e, dc, hk):
        # first expert's W2 lives in two dc-major half tiles; others are a
        # single hk-major tile
        if e == e_first:
            t = w2t[e][dc // (DCT // 2)]
            return t, (dc % (DCT // 2)) * (HMT * 128) + hk * 128
        return w2t[e], hk * D + dc * 128

    efr = slice(e_first * 128, (e_first + 1) * 128)
    import contextlib

    raw = contextlib.ExitStack()
    nc._raw_tensors_stack = raw  # keep allocations live for program lifetime
    warmr = raw.enter_context(nc.sbuf_tensor("warmr", [128, CHUNK], f16))
    warmps = raw.enter_context(nc.psum_tensor("warmps", [128, CHUNK], f32))

    # --- pre-TileContext HAM warmup, part 1: a few dummy matmuls get the PE
    # busy right after the runtime entry barrier.  Kept short so the Tensor
    # engine is not the long pole into the Tile entry barrier (which would
    # delay the input DMA issues).  The MMs read uninitialized SBUF into a
    # never-read PSUM bank — only the PE activity matters (clock-gate
    # release needs ~3.4us of cumulative busy).
    for i in range(4):
        nc.tensor.matmul(
            warmps[:, :],
            warmr[:, :128],
            warmr[:, :],
            start=(i == 0),
            stop=(i == 3),
        )

    with TileContext(nc) as tc:
        with (
            tc.tile_pool(name="w1p", bufs=1) as w1p,
            tc.tile_pool(name="w2p", bufs=1) as w2p,
            tc.tile_pool(name="xp", bufs=2) as xp,
            tc.tile_pool(name="hp", bufs=1) as hp,
            tc.tile_pool(name="yp", bufs=3) as yp,
            tc.tile_pool(name="cp", bufs=1) as cp,
            tc.tile_pool(name="ps1", bufs=3, space="PSUM") as ps1,
            tc.tile_pool(name="ps2", bufs=3, space="PSUM") as ps2,
        ):
            # --- b1 rides the scalar ring first (tiny, needed by first ACT)
            b1t = cp.tile([128, E * HMT], f32, tag="b1", name="b1t")
            nc.scalar.dma_start(out=b1t, in_=b1[:, :])

            # --- HAM warmup, part 2: keep the PE busy until the first real
            # data lands (combined with part 1, the ~3.4us busy window
            # releases the clock gate before chunk 0 starts)
            for i in range(WARM_MM - 4):
                nc.tensor.matmul(
                    warmps[:, :],
                    warmr[:, :128],
                    warmr[:, :],
                    start=(i == 0),
                    stop=(i == WARM_MM - 5),
                )

            # --- input stream: everything on the sync HWDGE ring, in
            # startup-ramp order: W1[first] halfA | x chunk0 | W1 halfB |
            # W2 halfA | W2 halfB | x chunk1 | ...
            w1t = [None] * E
            w2t = [None] * E
            w1ft = w1p.tile([128, DK * HS], f16, tag="w1f", name="w1ft")
            w1t[e_first] = w1ft
            nc.sync.dma_start(out=w1ft[:, :WHALF], in_=W1[efr, :WHALF])

            def load_w(e, which):
                if which == 1:
                    t = w1p.tile([128, DK * HS], f16, tag=f"w1_{e}", name=f"w1t{e}")
                    w1t[e] = t
                    src = W1[e * 128 : (e + 1) * 128, :]
                else:
                    t = w2p.tile([128, HMT * D], f16, tag=f"w2_{e}", name=f"w2t{e}")
                    w2t[e] = t
                    src = W2[e * 128 : (e + 1) * 128, :]
                nc.sync.dma_start(out=t, in_=src)

            def load_xg(gi):
                c0, ng = groups[gi]
                off = chunks[c0][1]
                gw = sum(chunks[c0 + j][2] for j in range(ng))
                gt = xp.tile([128, DK * xgrp_max], f16, tag="xg", name=f"xg{gi}")
                nc.sync.dma_start(
                    out=gt[:, : DK * gw], in_=xP[:, DK * off : DK * (off + gw)]
                )
                sub = 0
                for j in range(ng):
                    w = chunks[c0 + j][2]
                    xtiles[c0 + j] = [
                        gt[:, dk * gw + sub : dk * gw + sub + w] for dk in range(DK)
                    ]
                    sub += w

            xtiles = [None] * n_chunks
            load_xg(0)
            nc.sync.dma_start(out=w1ft[:, WHALF:], in_=W1[efr, WHALF:])
            # first expert's W2: two dc-major half tiles so gemm2 of chunk 0
            # can start on the first half while the second is in flight
            w2fa = w2p.tile([128, WHALF], f16, tag="w2fa", name="w2fa")
            nc.sync.dma_start(out=w2fa, in_=W2[efr, :WHALF])
            w2fb = w2p.tile([128, WHALF], f16, tag="w2fb", name="w2fb")
            nc.sync.dma_start(out=w2fb, in_=W2[efr, WHALF:])
            w2t[e_first] = (w2fa, w2fb)
            load_xg(1)

            # expert k's first chunk index
            estart = {}
            for ci, (e, off, w) in enumerate(chunks):
                estart.setdefault(e, ci)
            next_ei = 1  # index into `order`
            for gi in range(2, len(groups)):
                load_xg(gi)
                # issue weights for experts whose chunks begin within the
                # next couple of groups
                horizon = groups[min(gi + 2, len(groups) - 1)][0] + 1
                while next_ei < E and estart[order[next_ei]] <= horizon + 2:
                    load_w(order[next_ei], 1)
                    load_w(order[next_ei], 2)
                    next_ei += 1
            while next_ei < E:
                load_w(order[next_ei], 1)
                load_w(order[next_ei], 2)
                next_ei += 1

            for ci, (e, off, w) in enumerate(chunks):
                xt = xtiles[ci]
                ht = [
                    hp.tile([128, CHUNK], f16, tag=f"h{hm}", name=f"ht{hm}")
                    for hm in range(HMT)
                ]
                for hm in range(HMT):
                    ps = ps1.tile([128, CHUNK], f32, tag="ps1", name="ps1t")
                    for dk in range(DK):
                        nc.tensor.matmul(
                            ps[:, :w],
                            w1t[e][:, dk * HS + hm * 128 : dk * HS + (hm + 1) * 128],
                            xt[dk],
                            start=(dk == 0),
                            stop=(dk == DK - 1),
                        )
                    col = e * HMT + hm
                    nc.scalar.activation(
                        ht[hm][:, :w], ps[:, :w], Relu, bias=b1t[:, col : col + 1]
                    )
                # gemm2: yT[dc] = sum_hk W2.T @ h, packed into one y tile;
                # y written as two half DMAs on the scalar ring
                yt = yp.tile([128, DCT * CHUNK], f16, tag="yt", name="yt")
                half = DCT // 2
                for dh in range(2):
                    for dc in range(dh * half, (dh + 1) * half):
                        ps = ps2.tile([128, CHUNK], f32, tag="ps2", name="ps2t")
                        for hk in range(HMT):
                            w2s, col = w2tile(e, dc, hk)
                            nc.tensor.matmul(
                                ps[:, :w],
                                w2s[:, col : col + 128],
                                ht[hk][:, :w],
                                start=(hk == 0),
                                stop=(hk == HMT - 1),
                            )
                        nc.vector.tensor_copy(yt[:, dc * w : (dc + 1) * w], ps[:, :w])
                    nc.scalar.dma_start(
                        out=yP[
                            :, DCT * off + dh * half * w : DCT * off + (dh + 1) * half * w
                        ],
                        in_=yt[:, dh * half * w : (dh + 1) * half * w],
                    )
    nc.compile()
    return nc


def _get_nc(counts):
    if counts not in _NC_CACHE:
        _NC_CACHE[counts] = _build_nc(counts)
    return _NC_CACHE[counts]


def kernel(x, Wg, bg, W1, b1, W2, b2):
    global LAST_RESULTS
    x = np.asarray(x, dtype=np.float32)
    Wg = np.asarray(Wg, dtype=np.float32)
    bg = np.asarray(bg, dtype=np.float32)
    W1 = np.asarray(W1, dtype=np.float32)
    b1 = np.asarray(b1, dtype=np.float32)
    W2 = np.asarray(W2, dtype=np.float32)
    b2 = np.asarray(b2, dtype=np.float32)

    # --- gate + top-k routing (replicated small gate, on host) ---
    g = x @ Wg + bg  # [N, E]
    order_idx = np.argsort(-g, axis=1, kind="stable")[:, :TOPK]  # [N, 2]
    topv = np.take_along_axis(g, order_idx, axis=1)
    topv = topv - topv.max(axis=1, keepdims=True)
    ex = np.exp(topv)
    sw = ex / ex.sum(axis=1, keepdims=True)  # [N, 2] softmax over selected

    counts = tuple(int((order_idx == e).sum()) for e in range(E))
    nc = _get_nc(counts)
    eorder = _expert_order(counts)
    e_first = eorder[0]
    chunks = _chunks(counts)
    groups = _xgroups(chunks)

    # --- dispatch: expert-grouped pair order (in processing order),
    # replicated to all cores ---
    pos = np.empty((N_TOK, TOPK), np.int64)  # (token, k) -> pair column
    offs = {}
    off = 0
    toks = []
    for e in eorder:
        tok, kk = np.where(order_idx == e)
        pos[tok, kk] = off + np.arange(tok.size)
        offs[e] = off
        toks.append(tok)
        off += tok.size
    tok_all = np.concatenate(toks)
    xT = x[tok_all].T.astype(np.float16)  # [D, PAIRS]

    # pack x per chunk-group: [128, DK*gw] blocks, dk-major columns
    xPk = np.empty((128, DK * PAIRS), np.float16)
    for c0, ng in groups:
        o = chunks[c0][1]
        gw = sum(chunks[c0 + j][2] for j in range(ng))
        blk = xT[:, o : o + gw].reshape(DK, 128, gw).transpose(1, 0, 2)
        xPk[:, DK * o : DK * (o + gw)] = blk.reshape(128, DK * gw)

    in_maps = []
    for c in range(NCORES):
        sl = slice(c * HS, (c + 1) * HS)
        # pack each expert's weight slice as one [128, DK*HS] / [128, HMT*D]
        # row-block so it loads as a single fat-lined DMA descriptor
        W1s = np.ascontiguousarray(
            W1[:, :, sl]
            .reshape(E, DK, 128, HS)
            .transpose(0, 2, 1, 3)
            .reshape(E * 128, DK * HS)
        ).astype(np.float16)
        W2r = W2[:, sl, :].reshape(E, HMT, 128, D)
        W2s = np.empty((E, 128, HMT * D), np.float32)
        for e in range(E):
            if e == e_first:
                # dc-major block: [128, DCT * HMT * 128]
                W2s[e] = (
                    W2r[e]
                    .reshape(HMT, 128, DCT, 128)
                    .transpose(1, 2, 0, 3)
                    .reshape(128, DCT * HMT * 128)
                )
            else:
                W2s[e] = W2r[e].transpose(1, 0, 2).reshape(128, HMT * D)
        W2s = np.ascontiguousarray(W2s.reshape(E * 128, HMT * D)).astype(np.float16)
        b1s = np.ascontiguousarray(
            b1[:, sl].reshape(E, HMT, 128).transpose(2, 0, 1).reshape(128, E * HMT)
        )
        in_maps.append({"xP": xPk, "W1": W1s, "W2": W2s, "b1": b1s})

    kwargs = {}
    if TRACE_CORES is not None:
        kwargs["trace_cores"] = TRACE_CORES
    LAST_RESULTS = bass_utils.run_bass_kernel_spmd(
        nc, in_maps, core_ids=list(range(NCORES)), trace=TRACE, **kwargs
    )

    # --- combine: sum partials over cores, unpack, add b2, gate-weighted
    # scatter into the final [N, D] output ---
    Ps = np.zeros((128, DCT * PAIRS), np.float32)
    for r in LAST_RESULTS.results:
        Ps += r["yP"].astype(np.float32)
    Y = np.empty((PAIRS, D), np.float32)  # pair-major
    for e, off, w in chunks:
        blk = Ps[:, DCT * off : DCT * (off + w)].reshape(128, DCT, w)
        Y[off : off + w] = blk.transpose(1, 0, 2).reshape(D, w).T
    for e in eorder:
        n = counts[e]
        if np.any(b2[e]):
            Y[offs[e] : offs[e] + n] += b2[e][None, :]
    out = sw[:, 0, None] * Y[pos[:, 0]] + sw[:, 1, None] * Y[pos[:, 1]]
    return out.astype(np.float32)


# revision 28
# speedup vs baseline: 1.0559x; 1.0559x over previous
"""MoE layer (N=4096, D=1024, H=4096, E=8, top-2) on 8 Trainium2 cores.

Strategy: hidden-dim tensor-parallel.
  - Host computes the small gate, top-2 ids and softmax weights, groups the
    8192 (token, expert) pairs by expert, and replicates the grouped
    activations to all 8 cores.
  - Core c holds the hidden slice [c*512, (c+1)*512) of ALL experts' W1/W2
    (SBUF-resident, loaded once) and computes for every pair the partial
    FFN over its slice; host sums the 8 partial outputs, adds b2 and does
    the gate-weighted scatter into the [N, D] output.
  - Every core runs the exact same pair columns, so there is zero capacity
    padding and perfect load balance regardless of routing skew.

Device kernel (identical SPMD program on all 8 cores):
  - All matmul operands fp16 (bf16 PE rate, fp32 PSUM accumulation).
  - Pairs processed in expert-pure chunks of <=512 columns:
      gemm1: h[hm][:, chunk]  = relu(sum_dk W1t.T @ x)     (ACT drains PSUM)
      gemm2: yT[dc][:, chunk] = sum_hk W2t.T @ h           (DVE drains PSUM)
  - Ring plan (v2): ALL input streams (x groups + weights) ride the sync
    HWDGE ring; ALL y writes ride the scalar HWDGE ring.  gpsimd/SWDGE is
    not used for any steady-state DMA: its descriptor rings share SBUF AXI
    ports with SDMA engines 7/15, and a single straggling per-engine
    completion increment on the y semaphore was observed to stall the PE
    ~2.7us mid-kernel and re-throttle the HAM clock gate.
  - Startup ramp: the first expert processed has ~1024 pairs split
    [192, 320, rest]; its W1/W2 load as two half-tiles each so the first
    gemm1/gemm2 chains can start on the first half while the second is in
    flight (chunk-0 gemm1 runs dk 0-3 for an hm pair, then dk 4-7).  The
    first expert's W2 block is packed dc-major so gemm2 dc 0-3 only needs
    the first half.
  - Drain tail: the last expert ends with a 96-wide chunk; every chunk's y
    is written as two half DMAs (dc 0-3 after their casts, dc 4-7 after)
    so the post-compute drain is short.
"""

import numpy as np

from concourse import bacc
import concourse.mybir as mybir
from concourse.tile import TileContext
import concourse.bass_utils as bass_utils

N_TOK, D, H, E, TOPK = 4096, 1024, 4096, 8, 2
NCORES = 8
PAIRS = N_TOK * TOPK  # 8192 (token, expert) pairs, expert-grouped
HS = H // NCORES      # 512 hidden units per core
DK = D // 128         # 8 contraction tiles for gemm1
HMT = HS // 128       # 4 hidden tiles (gemm1 out / gemm2 contraction)
DCT = D // 128        # 8 output-column tiles for gemm2
CHUNK = 512           # max pair-chunk width (one fp32 PSUM bank)
XGW = 1024            # max packed-x-group width (SBUF budget cap)
WARM_MM = 11          # HAM warmup matmuls bridging entry -> first data
LEAD = (256, 320)     # startup ramp chunk widths for the first expert
TAIL_W = 96           # width of the final drain-friendly chunk

TRACE = False
TRACE_CORES = None
LAST_RESULTS = None

_NC_CACHE = {}


def _expert_order(counts):
    """First expert: count closest to LEAD-ramp-friendly 1024; last: smallest
    count (gets the TAIL_W drain chunk); middle: natural order."""
    first = min(range(E), key=lambda e: abs(counts[e] - 1024))
    rest = [e for e in range(E) if e != first]
    last = min(rest, key=lambda e: counts[e])
    mid = [e for e in rest if e != last]
    return [first] + mid + [last]


def _balanced(n, wmax=CHUNK):
    k = -(-n // wmax) if n else 0
    return [n // k + (1 if i < n % k else 0) for i in range(k)]


def _chunks(counts):
    """Expert-pure chunks of width <= CHUNK, with a startup ramp on the
    first expert and a short drain chunk on the last."""
    order = _expert_order(counts)
    out = []
    off = 0
    for i, e in enumerate(order):
        n = counts[e]
        if i == 0 and n > sum(LEAD) + 64:
            ws = list(LEAD) + _balanced(n - sum(LEAD))
        elif i == len(order) - 1 and n > TAIL_W + 128:
            ws = _balanced(n - TAIL_W) + [TAIL_W]
        else:
            ws = _balanced(n)
        for w in ws:
            out.append((e, off, w))
            off += w
    return out


def _xgroups(chunks):
    """Chunk groups for packed x descriptors: [c0], [c1], then greedy pairs
    capped at XGW total width (SBUF budget)."""
    groups = [(0, 1)]
    if len(chunks) > 1:
        groups.append((1, 1))
    i = 2
    while i < len(chunks):
        n = 1
        if i + 1 < len(chunks) and chunks[i][2] + chunks[i + 1][2] <= XGW:
            n = 2
        groups.append((i, n))
        i += n
    return groups


def _build_nc(counts):
    f16, f32 = mybir.dt.float16, mybir.dt.float32
    Relu = mybir.ActivationFunctionType.Relu
    nc = bacc.Bacc("TRN2", target_bir_lowering=False)
    xP = nc.dram_tensor("xP", [128, DK * PAIRS], f16, kind="ExternalInput")
    W1 = nc.dram_tensor("W1", [E * 128, DK * HS], f16, kind="ExternalInput")
    W2 = nc.dram_tensor("W2", [E * 128, HMT * D], f16, kind="ExternalInput")
    b1 = nc.dram_tensor("b1", [128, E * HMT], f32, kind="ExternalInput")
    yP = nc.dram_tensor("yP", [128, DCT * PAIRS], f16, kind="ExternalOutput")

    order = _expert_order(counts)
    e_first = order[0]
    chunks = _chunks(counts)
    groups = _xgroups(chunks)
    n_chunks = len(chunks)
    xgrp_max = max(sum(chunks[c0 + j][2] for j in range(ng)) for c0, ng in groups)
    WHALF = DK * HS // 2  # columns per W1/W2 half tile

    def w2tile(e, dc, hk):
        # first expert's W2 lives in two dc-major half tiles; others are a
        # single hk-major tile
        if e == e_first:
            t = w2t[e][dc // (DCT // 2)]
            return t, (dc % (DCT // 2)) * (HMT * 128) + hk * 128
        return w2t[e], hk * D + dc * 128

    efr = slice(e_first * 128, (e_first + 1) * 128)
    import contextlib

    raw = contextlib.ExitStack()
    nc._raw_tensors_stack = raw  # keep allocations live for program lifetime
    warmr = raw.enter_context(nc.sbuf_tensor("warmr", [128, CHUNK], f16))
    warmps = raw.enter_context(nc.psum_tensor("warmps", [128, CHUNK], f32))

    with TileContext(nc) as tc:
        with (
            tc.tile_pool(name="w1p", bufs=1) as w1p,
            tc.tile_pool(name="w2p", bufs=1) as w2p,
            tc.tile_pool(name="xp", bufs=2) as xp,
            tc.tile_pool(name="hp", bufs=1) as hp,
            tc.tile_pool(name="yp", bufs=3) as yp,
            tc.tile_pool(name="cp", bufs=1) as cp,
            tc.tile_pool(name="ps1", bufs=3, space="PSUM") as ps1,
            tc.tile_pool(name="ps2", bufs=3, space="PSUM") as ps2,
        ):
            # --- b1 rides the scalar ring first (tiny, needed by first ACT)
            b1t = cp.tile([128, E * HMT], f32, tag="b1", name="b1t")
            nc.scalar.dma_start(out=b1t, in_=b1[:, :])

            # --- HAM warmup: dummy matmuls keep the PE busy from Tile entry
            # until the first real data lands, releasing the clock gate
            # (~3.4us of busy) so chunk 0 runs at the full 2.4 GHz.  They
            # read the uninitialized raw warm tile (no memset dependency, so
            # they start immediately) into a never-read raw PSUM bank.
            for i in range(WARM_MM):
                nc.tensor.matmul(
                    warmps[:, :],
                    warmr[:, :128],
                    warmr[:, :],
                    start=(i == 0),
                    stop=(i == WARM_MM - 1),
                )

            # --- input stream: everything on the sync HWDGE ring, in
            # startup-ramp order: W1[first] halfA | x chunk0 | W1 halfB |
            # W2 halfA | W2 halfB | x chunk1 | ...
            w1t = [None] * E
            w2t = [None] * E
            w1ft = w1p.tile([128, DK * HS], f16, tag="w1f", name="w1ft")
            w1t[e_first] = w1ft
            nc.sync.dma_start(out=w1ft[:, :WHALF], in_=W1[efr, :WHALF])

            def load_w(e, which):
                if which == 1:
                    t = w1p.tile([128, DK * HS], f16, tag=f"w1_{e}", name=f"w1t{e}")
                    w1t[e] = t
                    src = W1[e * 128 : (e + 1) * 128, :]
                else:
                    t = w2p.tile([128, HMT * D], f16, tag=f"w2_{e}", name=f"w2t{e}")
                    w2t[e] = t
                    src = W2[e * 128 : (e + 1) * 128, :]
                nc.sync.dma_start(out=t, in_=src)

            def load_xg(gi):
                c0, ng = groups[gi]
                off = chunks[c0][1]
                gw = sum(chunks[c0 + j][2] for j in range(ng))
                gt = xp.tile([128, DK * xgrp_max], f16, tag="xg", name=f"xg{gi}")
                nc.sync.dma_start(
                    out=gt[:, : DK * gw], in_=xP[:, DK * off : DK * (off + gw)]
                )
                sub = 0
                for j in range(ng):
                    w = chunks[c0 + j][2]
                    xtiles[c0 + j] = [
                        gt[:, dk * gw + sub : dk * gw + sub + w] for dk in range(DK)
                    ]
                    sub += w

            xtiles = [None] * n_chunks
            load_xg(0)
            nc.sync.dma_start(out=w1ft[:, WHALF:], in_=W1[efr, WHALF:])
            # first expert's W2: two dc-major half tiles so gemm2 of chunk 0
            # can start on the first half while the second is in flight
            w2fa = w2p.tile([128, WHALF], f16, tag="w2fa", name="w2fa")
            nc.sync.dma_start(out=w2fa, in_=W2[efr, :WHALF])
            w2fb = w2p.tile([128, WHALF], f16, tag="w2fb", name="w2fb")
            nc.sync.dma_start(out=w2fb, in_=W2[efr, WHALF:])
            w2t[e_first] = (w2fa, w2fb)
            load_xg(1)

            # expert k's first chunk index
            estart = {}
            for ci, (e, off, w) in enumerate(chunks):
                estart.setdefault(e, ci)
            next_ei = 1  # index into `order`
            pending_w = []
            for gi in range(2, len(groups)):
                load_xg(gi)
                # queue weights for experts whose chunks begin within the
                # next couple of groups, but issue at most two weight loads
                # per group so x groups are never starved behind a burst of
                # weight traffic on the ring
                horizon = groups[min(gi + 2, len(groups) - 1)][0] + 1
                while next_ei < E and estart[order[next_ei]] <= horizon + 2:
                    pending_w.append((order[next_ei], 1))
                    pending_w.append((order[next_ei], 2))
                    next_ei += 1
                for _ in range(2):
                    if pending_w:
                        load_w(*pending_w.pop(0))
            for ew in pending_w:
                load_w(*ew)
            while next_ei < E:
                load_w(order[next_ei], 1)
                load_w(order[next_ei], 2)
                next_ei += 1

            for ci, (e, off, w) in enumerate(chunks):
                xt = xtiles[ci]
                ht = [
                    hp.tile([128, CHUNK], f16, tag=f"h{hm}", name=f"ht{hm}")
                    for hm in range(HMT)
                ]
                for hm in range(HMT):
                    ps = ps1.tile([128, CHUNK], f32, tag="ps1", name="ps1t")
                    for dk in range(DK):
                        nc.tensor.matmul(
                            ps[:, :w],
                            w1t[e][:, dk * HS + hm * 128 : dk * HS + (hm + 1) * 128],
                            xt[dk],
                            start=(dk == 0),
                            stop=(dk == DK - 1),
                        )
                    col = e * HMT + hm
                    nc.scalar.activation(
                        ht[hm][:, :w], ps[:, :w], Relu, bias=b1t[:, col : col + 1]
                    )
                # gemm2: yT[dc] = sum_hk W2.T @ h, packed into one y tile;
                # y written as two half DMAs on the scalar ring
                yt = yp.tile([128, DCT * CHUNK], f16, tag="yt", name="yt")
                half = DCT // 2
                for dh in range(2):
                    for dc in range(dh * half, (dh + 1) * half):
                        ps = ps2.tile([128, CHUNK], f32, tag="ps2", name="ps2t")
                        for hk in range(HMT):
                            w2s, col = w2tile(e, dc, hk)
                            nc.tensor.matmul(
                                ps[:, :w],
                                w2s[:, col : col + 128],
                                ht[hk][:, :w],
                                start=(hk == 0),
                                stop=(hk == HMT - 1),
                            )
                        nc.vector.tensor_copy(yt[:, dc * w : (dc + 1) * w], ps[:, :w])
                    nc.scalar.dma_start(
                        out=yP[
                            :, DCT * off + dh * half * w : DCT * off + (dh + 1) * half * w
                        ],
                        in_=yt[:, dh * half * w : (dh + 1) * half * w],
                    )
    nc.compile()
    return nc


def _get_nc(counts):
    if counts not in _NC_CACHE:
        _NC_CACHE[counts] = _build_nc(counts)
    return _NC_CACHE[counts]


def kernel(x, Wg, bg, W1, b1, W2, b2):
    global LAST_RESULTS
    x = np.asarray(x, dtype=np.float32)
    Wg = np.asarray(Wg, dtype=np.float32)
    bg = np.asarray(bg, dtype=np.float32)
    W1 = np.asarray(W1, dtype=np.float32)
    b1 = np.asarray(b1, dtype=np.float32)
    W2 = np.asarray(W2, dtype=np.float32)
    b2 = np.asarray(b2, dtype=np.float32)

    # --- gate + top-k routing (replicated small gate, on host) ---
    g = x @ Wg + bg  # [N, E]
    order_idx = np.argsort(-g, axis=1, kind="stable")[:, :TOPK]  # [N, 2]
    topv = np.take_along_axis(g, order_idx, axis=1)
    topv = topv - topv.max(axis=1, keepdims=True)
    ex = np.exp(topv)
    sw = ex / ex.sum(axis=1, keepdims=True)  # [N, 2] softmax over selected

    counts = tuple(int((order_idx == e).sum()) for e in range(E))
    nc = _get_nc(counts)
    eorder = _expert_order(counts)
    e_first = eorder[0]
    chunks = _chunks(counts)
    groups = _xgroups(chunks)

    # --- dispatch: expert-grouped pair order (in processing order),
    # replicated to all cores ---
    pos = np.empty((N_TOK, TOPK), np.int64)  # (token, k) -> pair column
    offs = {}
    off = 0
    toks = []
    for e in eorder:
        tok, kk = np.where(order_idx == e)
        pos[tok, kk] = off + np.arange(tok.size)
        offs[e] = off
        toks.append(tok)
        off += tok.size
    tok_all = np.concatenate(toks)
    xT = x[tok_all].T.astype(np.float16)  # [D, PAIRS]

    # pack x per chunk-group: [128, DK*gw] blocks, dk-major columns
    xPk = np.empty((128, DK * PAIRS), np.float16)
    for c0, ng in groups:
        o = chunks[c0][1]
        gw = sum(chunks[c0 + j][2] for j in range(ng))
        blk = xT[:, o : o + gw].reshape(DK, 128, gw).transpose(1, 0, 2)
        xPk[:, DK * o : DK * (o + gw)] = blk.reshape(128, DK * gw)

    in_maps = []
    for c in range(NCORES):
        sl = slice(c * HS, (c + 1) * HS)
        # pack each expert's weight slice as one [128, DK*HS] / [128, HMT*D]
        # row-block so it loads as a single fat-lined DMA descriptor
        W1s = np.ascontiguousarray(
            W1[:, :, sl]
            .reshape(E, DK, 128, HS)
            .transpose(0, 2, 1, 3)
            .reshape(E * 128, DK * HS)
        ).astype(np.float16)
        W2r = W2[:, sl, :].reshape(E, HMT, 128, D)
        W2s = np.empty((E, 128, HMT * D), np.float32)
        for e in range(E):
            if e == e_first:
                # dc-major block: [128, DCT * HMT * 128]
                W2s[e] = (
                    W2r[e]
                    .reshape(HMT, 128, DCT, 128)
                    .transpose(1, 2, 0, 3)
                    .reshape(128, DCT * HMT * 128)
                )
            else:
                W2s[e] = W2r[e].transpose(1, 0, 2).reshape(128, HMT * D)
        W2s = np.ascontiguousarray(W2s.reshape(E * 128, HMT * D)).astype(np.float16)
        b1s = np.ascontiguousarray(
            b1[:, sl].reshape(E, HMT, 128).transpose(2, 0, 1).reshape(128, E * HMT)
        )
        in_maps.append({"xP": xPk, "W1": W1s, "W2": W2s, "b1": b1s})

    kwargs = {}
    if TRACE_CORES is not None:
        kwargs["trace_cores"] = TRACE_CORES
    LAST_RESULTS = bass_utils.run_bass_kernel_spmd(
        nc, in_maps, core_ids=list(range(NCORES)), trace=TRACE, **kwargs
    )

    # --- combine: sum partials over cores, unpack, add b2, gate-weighted
    # scatter into the final [N, D] output ---
    Ps = np.zeros((128, DCT * PAIRS), np.float32)
    for r in LAST_RESULTS.results:
        Ps += r["yP"].astype(np.float32)
    Y = np.empty((PAIRS, D), np.float32)  # pair-major
    for e, off, w in chunks:
        blk = Ps[:, DCT * off : DCT * (off + w)].reshape(128, DCT, w)
        Y[off : off + w] = blk.transpose(1, 0, 2).reshape(D, w).T
    for e in eorder:
        n = counts[e]
        if np.any(b2[e]):
            Y[offs[e] : offs[e] + n] += b2[e][None, :]
    out = sw[:, 0, None] * Y[pos[:, 0]] + sw[:, 1, None] * Y[pos[:, 1]]
    return out.astype(np.float32)
